# revision 19
# baseline (speedup 1.0000x reference)
"""Trainium2 Bass kernel for DeepInstructedAttentionPositionScores.

Output [1, 8, 4116, 4116] f32 (~542 MB), one head per NeuronCore (8 cores).
Per core the output slab (67.8 MB) is pure HBM-write-bound, so the kernel is
built to keep ONE HWDGE store ring at full line rate (~620 ns per 16 KB
packet, ~420 GB/s measured) from ~23 us on:

  - four tiny matvecs on PE produce the h/w/d/cross score vectors;
  - the w/d Toeplitz windows are expanded by gathering small Hankel
    matrices H[k,a] = s[16+k+a] from a DRAM scratch row (positive-stride
    overlapping reads), then one PE matmul per half against constant 0/1
    selection matrices (iota-select) places the right window per group;
  - h/cross scores are broadcast across partitions with ones-matmuls on PE;
  - two persistent "V" buffers [128, 31*256] hold cb + hs for every stripe:
      V_b[p, 256*u + m] = cb_b[p, m] + hs[16 + u]
    so stripe content is a cheap DVE 2x-mode fp32 copy from a sliding
    4096-wide window of V_b into one of 6 rotating slot tiles whose cross
    columns [0:20] are pre-filled;
  - all 32 stores write full [128, 4116] contiguous rows on the SYNC ring
    only. Two concurrently-draining rings make the SDMA engines alternate
    between distant write streams (packet time 633 -> 927 ns measured), and
    sub-512B cross-column descriptors cost ~7 us/packet — both avoided.
  - no gpsimd/SWDGE DMAs at all.
"""
import numpy as np

DIMQ = 4116
DIMI = 20
C_CONT = 0.125 / 3.0
C_CROSS = 0.125
NSLOT = 6

_CACHE = {}
LAST_RESULTS = None


def _split_multi_waits(nc):
    """The walrus build in this image only encodes one semaphore wait per
    instruction. Tile emits multi-wait sync_info; split the extras into
    single-wait Drain instructions inserted just before, on the same engine
    (program order preserved, so semantics are unchanged)."""
    import concourse.mybir as mybir

    fn = nc.m.functions[0]
    ctr = 0
    for blk in fn.blocks:
        out = []
        for inst in blk.instructions:
            si = inst.sync_info
            waits = list(si.on_wait) if (si is not None and si.on_wait) else []
            if len(waits) > 1:
                for w in waits[:-1]:
                    ctr += 1
                    d = mybir.InstDrain(name=f"msw-{ctr}", ins=[], outs=[])
                    d.engine = inst.engine
                    d.sync_info = mybir.SyncInfo(on_wait=[w], on_update=[])
                    out.append(d)
                si.on_wait = waits[-1:]
            out.append(inst)
        blk.instructions = out
    return nc


def _build_nc(split=True):
    import concourse.bass as bass
    import concourse.mybir as mybir
    import concourse.tile as tile
    from contextlib import ExitStack

    DT = mybir.dt.float32
    nc = bass.Bass()
    tabw_d = nc.dram_tensor("tabw", [64, 213], DT, kind="ExternalInput")
    out_d = nc.dram_tensor("out", [DIMQ, DIMQ], DT, kind="ExternalOutput")
    # packed score row: ws*C (0:63) | ds*C (63:126) | cs*CC (126:146)
    scr_wd = nc.dram_tensor("scr_wd", [1, 146], DT)

    with tile.TileContext(nc) as tc:
        with ExitStack() as ctx:
            const = ctx.enter_context(tc.tile_pool(name="const", bufs=1))
            psum = ctx.enter_context(tc.tile_pool(name="psum", bufs=1, space="PSUM"))

            # ---- input table (sync ring) ----
            tabw = const.tile([64, 213], DT)
            nc.sync.dma_start(tabw[:], tabw_d[:])

            # ---- slot tiles; slot 5 doubles as the zero source ----
            slots = [const.tile([128, DIMQ], DT, tag=f"slot{s}", name=f"slot{s}")
                     for s in range(NSLOT)]
            nc.vector.memset(slots[NSLOT - 1][:], 0.0)
            ones = const.tile([1, 128], DT)
            nc.vector.memset(ones[:], 1.0)

            # ---- constant 0/1 selection matrices on pool (no deps) ----
            # A_w0[k, 16g+q] = 1 iff k == 15 - g ; A_w1: k == 7 - g ;
            # A_d[k, 16g+q] = 1 iff k == 15 - q.
            eq = mybir.AluOpType.is_equal
            a_w0 = const.tile([16, 128], DT)
            nc.gpsimd.memset(a_w0[:], 1.0)
            nc.gpsimd.affine_select(out=a_w0[:], in_=a_w0[:], fill=0.0, base=-15,
                                    channel_multiplier=1, compare_op=eq,
                                    pattern=[[1, 8], [0, 16]])
            a_w1 = const.tile([16, 128], DT)
            nc.gpsimd.memset(a_w1[:], 1.0)
            nc.gpsimd.affine_select(out=a_w1[:], in_=a_w1[:], fill=0.0, base=-7,
                                    channel_multiplier=1, compare_op=eq,
                                    pattern=[[1, 8], [0, 16]])
            a_d = const.tile([16, 128], DT)
            nc.gpsimd.memset(a_d[:], 1.0)
            nc.gpsimd.affine_select(out=a_d[:], in_=a_d[:], fill=0.0, base=-15,
                                    channel_multiplier=1, compare_op=eq,
                                    pattern=[[0, 8], [1, 16]])

            # ---- tiny matvecs on PE ----
            # hs_row[u] = sum_c w_h[c] * enc_h[16+u, c]   (u = 0..30)
            p_hs = psum.tile([1, 31], DT, tag="p_hs")
            nc.tensor.matmul(p_hs[:], tabw[:, 209:210], tabw[:, 16:47])
            p_ws = psum.tile([1, 63], DT, tag="p_ws")
            nc.tensor.matmul(p_ws[:], tabw[:, 210:211], tabw[:, 63:126])
            p_ds = psum.tile([1, 63], DT, tag="p_ds")
            nc.tensor.matmul(p_ds[:], tabw[:, 211:212], tabw[:, 126:189])
            p_cs = psum.tile([1, 20], DT, tag="p_cs")
            nc.tensor.matmul(p_cs[:], tabw[:, 212:213], tabw[:, 189:209])

            # scaled copies PSUM -> SBUF (ACT)
            wd_sb = const.tile([1, 146], DT)
            nc.scalar.mul(wd_sb[:, 0:63], p_ws[:], C_CONT)
            nc.scalar.mul(wd_sb[:, 63:126], p_ds[:], C_CONT)
            nc.scalar.mul(wd_sb[:, 126:146], p_cs[:], C_CROSS)
            hs_row = const.tile([1, 31], DT)
            nc.scalar.mul(hs_row[:], p_hs[:], C_CONT)

            # broadcast hs / cross across partitions via ones-matmul on PE
            p_hb = psum.tile([128, 31], DT, tag="p_hb")
            nc.tensor.matmul(p_hb[:], ones[:], hs_row[:])
            p_cb = psum.tile([128, 20], DT, tag="p_cs", name="p_cb")
            nc.tensor.matmul(p_cb[:], ones[:], wd_sb[:, 126:146])

            # pack row to DRAM (ACT queue; issue is async, completion via sem)
            nc.scalar.dma_start(scr_wd[:], wd_sb[:])
            hs_sb = const.tile([128, 31], DT)
            nc.scalar.mul(hs_sb[:], p_hb[:], 1.0)
            cs_bc = const.tile([128, 20], DT)
            nc.scalar.mul(cs_bc[:], p_cb[:], 1.0)

            # Hankel gathers: H_w[k, a] = ws[16+k+a]*C ; H_d[k, m] = ds[16+k+m]*C
            h_w = const.tile([16, 16], DT)
            nc.sync.dma_start(h_w[:], bass.AP(scr_wd, 16, [[1, 16], [1, 16]]))
            h_d = const.tile([16, 16], DT)
            nc.scalar.dma_start(h_d[:], bass.AP(scr_wd, 79, [[1, 16], [1, 16]]))

            # zero rows 0..19 from the zeroed slot (sync ring, early)
            nc.sync.dma_start(out_d[0:DIMI, :], slots[NSLOT - 1][0:DIMI, :])

            # ---- expand via PE: wexp_b[p, a] = ws[31-i2(p)+a], etc ----
            p_w0 = psum.tile([128, 16], DT, tag="p_hs", name="p_w0")
            nc.tensor.matmul(p_w0[:], a_w0[:], h_w[:])
            p_w1 = psum.tile([128, 16], DT, tag="p_ws", name="p_w1")
            nc.tensor.matmul(p_w1[:], a_w1[:], h_w[:])
            p_d = psum.tile([128, 16], DT, tag="p_ds", name="p_d")
            nc.tensor.matmul(p_d[:], a_d[:], h_d[:])
            wexp0 = const.tile([128, 16], DT)
            nc.scalar.mul(wexp0[:], p_w0[:], 1.0)
            wexp1 = const.tile([128, 16], DT)
            nc.scalar.mul(wexp1[:], p_w1[:], 1.0)
            dexp = const.tile([128, 16], DT)
            nc.scalar.mul(dexp[:], p_d[:], 1.0)

            # cross columns into slots 0..4 (slot 5 after its zeros store)
            for s in range(NSLOT - 1):
                nc.vector.tensor_copy(slots[s][:, 0:DIMI], cs_bc[:])

            # ---- cb halves: cb_b[p, 16a+m] = wexp_b[p, a] + dexp[p, m] ----
            da = dexp[:]
            d_rep = bass.AP(da.tensor, da.offset, [[16, 128], [0, 16], [1, 16]])
            cbs = []
            for b, (wt, eng) in enumerate([(wexp0, nc.vector), (wexp1, nc.gpsimd)]):
                cb = const.tile([128, 256], DT, tag=f"cb{b}", name=f"cb{b}")
                wa = wt[:]
                w_exp = bass.AP(wa.tensor, wa.offset, [[16, 128], [1, 16], [0, 16]])
                eng.tensor_add(cb[:], w_exp, d_rep)
                cbs.append(cb)

            # ---- V buffers: V_b[p, 256u+m] = cb_b[p, m] + hs_sb[p, u] ----
            # Built on the POOL engine (2x slower than DVE but otherwise
            # idle) while DVE computes the first 8 stripes directly.
            vs = []
            for b in range(2):
                vs.append(const.tile([128, 7936], DT, tag=f"v{b}", name=f"v{b}"))
            ha = hs_sb[:]
            for (b, u0, nb) in [(0, 15, 16), (0, 0, 15), (1, 15, 16), (1, 0, 15)]:
                ca = cbs[b][:]
                cb_rep = bass.AP(ca.tensor, ca.offset, [[256, 128], [0, nb], [1, 256]])
                hs_rep = bass.AP(ha.tensor, ha.offset + u0, [[31, 128], [1, nb], [0, 256]])
                nc.gpsimd.tensor_add(vs[b][:, 256 * u0:256 * (u0 + nb)], cb_rep, hs_rep)

            def stripe_add(k):
                # direct cb + hs broadcast-add into the slot (no V needed)
                i1, b = k >> 1, k & 1
                ca = cbs[b][:]
                cb_rep = bass.AP(ca.tensor, ca.offset, [[256, 128], [0, 16], [1, 256]])
                hs_rep = bass.AP(ha.tensor, ha.offset + 15 - i1, [[31, 128], [1, 16], [0, 256]])
                nc.vector.tensor_add(slots[k % NSLOT][:, DIMI:DIMQ], cb_rep, hs_rep)

            def stripe_copy(k):
                # 2x-mode fp32 copy from the sliding V window
                i1, b = k >> 1, k & 1
                off = 256 * (15 - i1)
                va = vs[b][:]
                src = bass.AP(va.tensor, va.offset + off, [[7936, 128], [1, 4096]])
                nc.vector.tensor_copy(slots[k % NSLOT][:, DIMI:DIMQ], src)

            def stripe_store(k):
                r0 = DIMI + 128 * k
                nc.sync.dma_start(out_d[r0:r0 + 128, :], slots[k % NSLOT][:])

            # copy_k must precede store_k (and follow store_{k-NSLOT}) in
            # PROGRAM order — Tile's dependency tracking is program-order
            # based. First 8 stripes as direct adds (DVE, ~5.3 us each, no V
            # dependency); the rest as V-window copies once pool has built V.
            for k in range(32):
                if k == 5:
                    # slot5 cross columns (zeros store has completed by now)
                    nc.vector.tensor_copy(slots[NSLOT - 1][:, 0:DIMI], cs_bc[:])
                if k < 8:
                    stripe_add(k)
                else:
                    stripe_copy(k)
                stripe_store(k)
    return _split_multi_waits(nc) if split else nc


def kernel(enc_cross, enc_h, enc_w, enc_d, w_cross, w_h, w_w, w_d,
           dim_q=4116, dim_k=4116, dim_i=20, dim_h=16, dim_w=16, dim_d=16,
           **_ignored):
    import os
    from concourse.bass_utils import run_bass_kernel_spmd
    global LAST_RESULTS

    enc_cross = np.asarray(enc_cross, dtype=np.float32)
    enc_h = np.asarray(enc_h, dtype=np.float32)
    enc_w = np.asarray(enc_w, dtype=np.float32)
    enc_d = np.asarray(enc_d, dtype=np.float32)
    w_cross = np.asarray(w_cross, dtype=np.float32)
    w_h = np.asarray(w_h, dtype=np.float32)
    w_w = np.asarray(w_w, dtype=np.float32)
    w_d = np.asarray(w_d, dtype=np.float32)

    cross2d = np.ascontiguousarray(enc_cross.reshape(DIMI, 64))
    tab = np.concatenate([enc_h.T, enc_w.T, enc_d.T, cross2d.T], axis=1)

    if "nc" not in _CACHE:
        _CACHE["nc"] = _build_nc()
    nc = _CACHE["nc"]

    core_ids = list(range(8))
    in_maps = []
    for h in core_ids:
        wp = np.stack([w_h[h], w_w[h], w_d[h], w_cross[h]], axis=1)
        tabw = np.ascontiguousarray(np.concatenate([tab, wp], axis=1))
        in_maps.append({"tabw": tabw})

    trace = bool(int(os.environ.get("KERNEL_TRACE", "0")))
    LAST_RESULTS = run_bass_kernel_spmd(nc, in_maps, core_ids, trace=trace)
    out = np.stack([LAST_RESULTS.results[h]["out"] for h in core_ids], axis=0)
    return out[None]


# revision 20
# speedup vs baseline: 1.1020x; 1.1020x over previous
"""Trainium2 Bass kernel for DeepInstructedAttentionPositionScores.

Output [1, 8, 4116, 4116] f32 (~542 MB), one head per NeuronCore (8 cores).
Per core the output slab (67.8 MB) is pure HBM-write-bound, so the kernel is
built to keep ONE HWDGE store ring at full line rate (~620 ns per 16 KB
packet, ~420 GB/s measured) from ~23 us on:

  - four tiny matvecs on PE produce the h/w/d/cross score vectors;
  - the w/d Toeplitz windows are expanded by gathering small Hankel
    matrices H[k,a] = s[16+k+a] from a DRAM scratch row (positive-stride
    overlapping reads), then one PE matmul per half against constant 0/1
    selection matrices (iota-select) places the right window per group;
  - h/cross scores are broadcast across partitions with ones-matmuls on PE;
  - two persistent "V" buffers [128, 31*256] hold cb + hs for every stripe:
      V_b[p, 256*u + m] = cb_b[p, m] + hs[16 + u]
    so stripe content is a cheap DVE 2x-mode fp32 copy from a sliding
    4096-wide window of V_b into one of 6 rotating slot tiles whose cross
    columns [0:20] are pre-filled;
  - all 32 stores write full [128, 4116] contiguous rows on the SYNC ring
    only. Two concurrently-draining rings make the SDMA engines alternate
    between distant write streams (packet time 633 -> 927 ns measured), and
    sub-512B cross-column descriptors cost ~7 us/packet — both avoided.
  - no gpsimd/SWDGE DMAs at all.
"""
import numpy as np

DIMQ = 4116
DIMI = 20
C_CONT = 0.125 / 3.0
C_CROSS = 0.125
NSLOT = 6

_CACHE = {}
LAST_RESULTS = None


def _split_multi_waits(nc):
    """The walrus build in this image only encodes one semaphore wait per
    instruction. Tile emits multi-wait sync_info; split the extras into
    single-wait Drain instructions inserted just before, on the same engine
    (program order preserved, so semantics are unchanged)."""
    import concourse.mybir as mybir

    fn = nc.m.functions[0]
    ctr = 0
    for blk in fn.blocks:
        out = []
        for inst in blk.instructions:
            si = inst.sync_info
            waits = list(si.on_wait) if (si is not None and si.on_wait) else []
            if len(waits) > 1:
                for w in waits[:-1]:
                    ctr += 1
                    d = mybir.InstDrain(name=f"msw-{ctr}", ins=[], outs=[])
                    d.engine = inst.engine
                    d.sync_info = mybir.SyncInfo(on_wait=[w], on_update=[])
                    out.append(d)
                si.on_wait = waits[-1:]
            out.append(inst)
        blk.instructions = out
    return nc


def _build_nc(split=True):
    import concourse.bass as bass
    import concourse.mybir as mybir
    import concourse.tile as tile
    from contextlib import ExitStack

    DT = mybir.dt.float32
    nc = bass.Bass()
    tabw_d = nc.dram_tensor("tabw", [64, 213], DT, kind="ExternalInput")
    out_d = nc.dram_tensor("out", [DIMQ, DIMQ], DT, kind="ExternalOutput")
    # packed score row: ws*C (0:63) | ds*C (63:126) | cs*CC (126:146)
    scr_wd = nc.dram_tensor("scr_wd", [1, 146], DT)

    with tile.TileContext(nc) as tc:
        with ExitStack() as ctx:
            const = ctx.enter_context(tc.tile_pool(name="const", bufs=1))
            psum = ctx.enter_context(tc.tile_pool(name="psum", bufs=1, space="PSUM"))

            # ---- input table (sync ring) ----
            tabw = const.tile([64, 213], DT)
            nc.sync.dma_start(tabw[:], tabw_d[:])

            # ---- slot tiles; slot 5 doubles as the zero source ----
            slots = [const.tile([128, DIMQ], DT, tag=f"slot{s}", name=f"slot{s}")
                     for s in range(NSLOT)]
            nc.vector.memset(slots[NSLOT - 1][:], 0.0)
            ones = const.tile([1, 128], DT)
            nc.vector.memset(ones[:], 1.0)

            # ---- constant 0/1 selection matrices on pool (no deps) ----
            # A_w0[k, 16g+q] = 1 iff k == 15 - g ; A_w1: k == 7 - g ;
            # A_d[k, 16g+q] = 1 iff k == 15 - q.
            eq = mybir.AluOpType.is_equal
            a_w0 = const.tile([16, 128], DT)
            nc.gpsimd.memset(a_w0[:], 1.0)
            nc.gpsimd.affine_select(out=a_w0[:], in_=a_w0[:], fill=0.0, base=-15,
                                    channel_multiplier=1, compare_op=eq,
                                    pattern=[[1, 8], [0, 16]])
            a_w1 = const.tile([16, 128], DT)
            nc.gpsimd.memset(a_w1[:], 1.0)
            nc.gpsimd.affine_select(out=a_w1[:], in_=a_w1[:], fill=0.0, base=-7,
                                    channel_multiplier=1, compare_op=eq,
                                    pattern=[[1, 8], [0, 16]])
            a_d = const.tile([16, 128], DT)
            nc.gpsimd.memset(a_d[:], 1.0)
            nc.gpsimd.affine_select(out=a_d[:], in_=a_d[:], fill=0.0, base=-15,
                                    channel_multiplier=1, compare_op=eq,
                                    pattern=[[0, 8], [1, 16]])

            # ---- tiny matvecs on PE ----
            # hs_row[u] = sum_c w_h[c] * enc_h[16+u, c]   (u = 0..30)
            p_hs = psum.tile([1, 31], DT, tag="p_hs")
            nc.tensor.matmul(p_hs[:], tabw[:, 209:210], tabw[:, 16:47])
            p_ws = psum.tile([1, 63], DT, tag="p_ws")
            nc.tensor.matmul(p_ws[:], tabw[:, 210:211], tabw[:, 63:126])
            p_ds = psum.tile([1, 63], DT, tag="p_ds")
            nc.tensor.matmul(p_ds[:], tabw[:, 211:212], tabw[:, 126:189])
            p_cs = psum.tile([1, 20], DT, tag="p_cs")
            nc.tensor.matmul(p_cs[:], tabw[:, 212:213], tabw[:, 189:209])

            # scaled copies PSUM -> SBUF (ACT)
            wd_sb = const.tile([1, 146], DT)
            nc.scalar.mul(wd_sb[:, 0:63], p_ws[:], C_CONT)
            nc.scalar.mul(wd_sb[:, 63:126], p_ds[:], C_CONT)
            nc.scalar.mul(wd_sb[:, 126:146], p_cs[:], C_CROSS)
            hs_row = const.tile([1, 31], DT)
            nc.scalar.mul(hs_row[:], p_hs[:], C_CONT)

            # broadcast hs / cross across partitions via ones-matmul on PE
            p_hb = psum.tile([128, 31], DT, tag="p_hb")
            nc.tensor.matmul(p_hb[:], ones[:], hs_row[:])
            p_cb = psum.tile([128, 20], DT, tag="p_cs", name="p_cb")
            nc.tensor.matmul(p_cb[:], ones[:], wd_sb[:, 126:146])

            # pack row to DRAM (ACT queue; issue is async, completion via sem)
            nc.scalar.dma_start(scr_wd[:], wd_sb[:])
            hs_sb = const.tile([128, 31], DT)
            nc.scalar.mul(hs_sb[:], p_hb[:], 1.0)
            cs_bc = const.tile([128, 20], DT)
            nc.scalar.mul(cs_bc[:], p_cb[:], 1.0)

            # Hankel gathers: H_w[k, a] = ws[16+k+a]*C ; H_d[k, m] = ds[16+k+m]*C
            h_w = const.tile([16, 16], DT)
            nc.sync.dma_start(h_w[:], bass.AP(scr_wd, 16, [[1, 16], [1, 16]]))
            h_d = const.tile([16, 16], DT)
            nc.scalar.dma_start(h_d[:], bass.AP(scr_wd, 79, [[1, 16], [1, 16]]))

            # zero rows 0..19 from the zeroed slot (sync ring, early)
            nc.sync.dma_start(out_d[0:DIMI, :], slots[NSLOT - 1][0:DIMI, :])

            # ---- expand via PE: wexp_b[p, a] = ws[31-i2(p)+a], etc ----
            p_w0 = psum.tile([128, 16], DT, tag="p_hs", name="p_w0")
            nc.tensor.matmul(p_w0[:], a_w0[:], h_w[:])
            p_w1 = psum.tile([128, 16], DT, tag="p_ws", name="p_w1")
            nc.tensor.matmul(p_w1[:], a_w1[:], h_w[:])
            p_d = psum.tile([128, 16], DT, tag="p_ds", name="p_d")
            nc.tensor.matmul(p_d[:], a_d[:], h_d[:])
            wexp0 = const.tile([128, 16], DT)
            nc.scalar.mul(wexp0[:], p_w0[:], 1.0)
            wexp1 = const.tile([128, 16], DT)
            nc.scalar.mul(wexp1[:], p_w1[:], 1.0)
            dexp = const.tile([128, 16], DT)
            nc.scalar.mul(dexp[:], p_d[:], 1.0)

            # cross columns into slots 0..4 (slot 5 after its zeros store)
            for s in range(NSLOT - 1):
                nc.vector.tensor_copy(slots[s][:, 0:DIMI], cs_bc[:])

            # ---- cb halves: cb_b[p, 16a+m] = wexp_b[p, a] + dexp[p, m] ----
            da = dexp[:]
            d_rep = bass.AP(da.tensor, da.offset, [[16, 128], [0, 16], [1, 16]])
            cbs = []
            for b, (wt, eng) in enumerate([(wexp0, nc.vector), (wexp1, nc.gpsimd)]):
                cb = const.tile([128, 256], DT, tag=f"cb{b}", name=f"cb{b}")
                wa = wt[:]
                w_exp = bass.AP(wa.tensor, wa.offset, [[16, 128], [1, 16], [0, 16]])
                eng.tensor_add(cb[:], w_exp, d_rep)
                cbs.append(cb)

            # ---- per-stripe content: ONE DVE broadcast-add into the slot:
            # slot[p, 20 + 256*j1 + m] = cb_b[p, m] + hs_sb[p, 15 - i1 + j1]
            # Measured 4.42 us per add — below the 4.96 us store drain
            # cadence, so the DVE keeps the ring sated. Nothing else runs
            # on vector/pool concurrently (a concurrent big pool tensor_add
            # thrashes DVE: 4.42 -> 11.3 us measured).
            ha = hs_sb[:]

            def stripe_add(k):
                i1, b = k >> 1, k & 1
                ca = cbs[b][:]
                cb_rep = bass.AP(ca.tensor, ca.offset, [[256, 128], [0, 16], [1, 256]])
                hs_rep = bass.AP(ha.tensor, ha.offset + 15 - i1, [[31, 128], [1, 16], [0, 256]])
                nc.vector.tensor_add(slots[k % NSLOT][:, DIMI:DIMQ], cb_rep, hs_rep)

            def stripe_store(k):
                r0 = DIMI + 128 * k
                nc.sync.dma_start(out_d[r0:r0 + 128, :], slots[k % NSLOT][:])

            # add_k must precede store_k (and follow store_{k-NSLOT}) in
            # PROGRAM order — Tile's dependency tracking is program-order
            # based.
            for k in range(32):
                if k == 5:
                    # slot5 cross columns (zeros store has completed by now)
                    nc.vector.tensor_copy(slots[NSLOT - 1][:, 0:DIMI], cs_bc[:])
                stripe_add(k)
                stripe_store(k)
    return _split_multi_waits(nc) if split else nc


def kernel(enc_cross, enc_h, enc_w, enc_d, w_cross, w_h, w_w, w_d,
           dim_q=4116, dim_k=4116, dim_i=20, dim_h=16, dim_w=16, dim_d=16,
           **_ignored):
    import os
    from concourse.bass_utils import run_bass_kernel_spmd
    global LAST_RESULTS

    enc_cross = np.asarray(enc_cross, dtype=np.float32)
    enc_h = np.asarray(enc_h, dtype=np.float32)
    enc_w = np.asarray(enc_w, dtype=np.float32)
    enc_d = np.asarray(enc_d, dtype=np.float32)
    w_cross = np.asarray(w_cross, dtype=np.float32)
    w_h = np.asarray(w_h, dtype=np.float32)
    w_w = np.asarray(w_w, dtype=np.float32)
    w_d = np.asarray(w_d, dtype=np.float32)

    cross2d = np.ascontiguousarray(enc_cross.reshape(DIMI, 64))
    tab = np.concatenate([enc_h.T, enc_w.T, enc_d.T, cross2d.T], axis=1)

    if "nc" not in _CACHE:
        _CACHE["nc"] = _build_nc()
    nc = _CACHE["nc"]

    core_ids = list(range(8))
    in_maps = []
    for h in core_ids:
        wp = np.stack([w_h[h], w_w[h], w_d[h], w_cross[h]], axis=1)
        tabw = np.ascontiguousarray(np.concatenate([tab, wp], axis=1))
        in_maps.append({"tabw": tabw})

    trace = bool(int(os.environ.get("KERNEL_TRACE", "0")))
    LAST_RESULTS = run_bass_kernel_spmd(nc, in_maps, core_ids, trace=trace)
    out = np.stack([LAST_RESULTS.results[h]["out"] for h in core_ids], axis=0)
    return out[None]


# revision 26
# speedup vs baseline: 1.3453x; 1.2208x over previous
"""Trainium2 Bass kernel for DeepInstructedAttentionPositionScores.

Output [1, 8, 4116, 4116] f32 (~542 MB), one head per NeuronCore (8 cores).
Per core the output slab (67.8 MB) is pure HBM-write-bound, so the kernel is
built to keep ONE HWDGE store ring at full line rate (~620 ns per 16 KB
packet, ~420 GB/s measured) from ~23 us on:

  - four tiny matvecs on PE produce the h/w/d/cross score vectors;
  - the w/d Toeplitz windows are expanded by gathering small Hankel
    matrices H[k,a] = s[16+k+a] from a DRAM scratch row (positive-stride
    overlapping reads), then one PE matmul per half against constant 0/1
    selection matrices (iota-select) places the right window per group;
  - h/cross scores are broadcast across partitions with ones-matmuls on PE;
  - each stripe's content is ONE DVE broadcast-add (4.42 us, below the
    ~4.96 us drain cadence) into one of 6 rotating slot tiles whose cross
    columns [0:20] are pre-filled:
      slot[p, 20 + 256*j1 + m] = cb_b[p, m] + hs_sb[p, 15 - i1 + j1]
  - all 32 stores write full [128, 4116] contiguous rows on the SYNC ring
    only. Two concurrently-draining rings make the SDMA engines alternate
    between distant write streams (packet time 633 -> 927 ns measured),
    sub-512B cross-column descriptors cost ~7 us/packet, and a big
    concurrent pool-engine tensor op thrashes the DVE (4.4 -> 11 us) —
    all three avoided. No gpsimd/SWDGE DMAs at all.
"""
import numpy as np

DIMQ = 4116
DIMI = 20
C_CONT = 0.125 / 3.0
C_CROSS = 0.125
NSLOT = 6

_CACHE = {}
LAST_RESULTS = None


def _split_multi_waits(nc):
    """The walrus build in this image only encodes one semaphore wait per
    instruction. Tile emits multi-wait sync_info; split the extras into
    single-wait Drain instructions inserted just before, on the same engine
    (program order preserved, so semantics are unchanged)."""
    import concourse.mybir as mybir

    fn = nc.m.functions[0]
    ctr = 0
    for blk in fn.blocks:
        out = []
        for inst in blk.instructions:
            si = inst.sync_info
            waits = list(si.on_wait) if (si is not None and si.on_wait) else []
            if len(waits) > 1:
                for w in waits[:-1]:
                    ctr += 1
                    d = mybir.InstDrain(name=f"msw-{ctr}", ins=[], outs=[])
                    d.engine = inst.engine
                    d.sync_info = mybir.SyncInfo(on_wait=[w], on_update=[])
                    out.append(d)
                si.on_wait = waits[-1:]
            out.append(inst)
        blk.instructions = out
    return nc


def _build_nc(split=True):
    import concourse.bass as bass
    import concourse.mybir as mybir
    import concourse.tile as tile
    from contextlib import ExitStack

    DT = mybir.dt.float32
    nc = bass.Bass()
    tabw_d = nc.dram_tensor("tabw", [64, 213], DT, kind="ExternalInput")
    out_d = nc.dram_tensor("out", [DIMQ, DIMQ], DT, kind="ExternalOutput")

    with tile.TileContext(nc) as tc:
        with ExitStack() as ctx:
            const = ctx.enter_context(tc.tile_pool(name="const", bufs=1))
            psum = ctx.enter_context(tc.tile_pool(name="psum", bufs=1, space="PSUM"))

            # ---- input table (sync ring) ----
            tabw = const.tile([64, 213], DT)
            nc.sync.dma_start(tabw[:], tabw_d[:])

            # ---- slot tiles; slot 5 doubles as the zero source ----
            slots = [const.tile([128, DIMQ], DT, tag=f"slot{s}", name=f"slot{s}")
                     for s in range(NSLOT)]
            ones = const.tile([1, 128], DT)
            nc.vector.memset(ones[:], 1.0)
            nc.vector.memset(slots[NSLOT - 1][:], 0.0)

            # ---- constant 0/1 selection matrices on pool (no deps) ----
            # A_w0[k, 16g+q] = 1 iff k == 15 - g ; A_w1: k == 7 - g ;
            # A_d[k, 16g+q] = 1 iff k == 15 - q.
            eq = mybir.AluOpType.is_equal
            a_w0 = const.tile([16, 128], DT)
            nc.gpsimd.memset(a_w0[:], 1.0)
            nc.gpsimd.affine_select(out=a_w0[:], in_=a_w0[:], fill=0.0, base=-15,
                                    channel_multiplier=1, compare_op=eq,
                                    pattern=[[1, 8], [0, 16]])
            a_w1 = const.tile([16, 128], DT)
            nc.gpsimd.memset(a_w1[:], 1.0)
            nc.gpsimd.affine_select(out=a_w1[:], in_=a_w1[:], fill=0.0, base=-7,
                                    channel_multiplier=1, compare_op=eq,
                                    pattern=[[1, 8], [0, 16]])
            a_d = const.tile([16, 128], DT)
            nc.gpsimd.memset(a_d[:], 1.0)
            nc.gpsimd.affine_select(out=a_d[:], in_=a_d[:], fill=0.0, base=-15,
                                    channel_multiplier=1, compare_op=eq,
                                    pattern=[[0, 8], [1, 16]])

            # ---- tiny matvecs on PE ----
            # hs_row[u] = sum_c w_h[c] * enc_h[16+u, c]   (u = 0..30)
            p_hs = psum.tile([1, 31], DT, tag="p_hs")
            nc.tensor.matmul(p_hs[:], tabw[:, 209:210], tabw[:, 16:47])
            p_ws = psum.tile([1, 63], DT, tag="p_ws")
            nc.tensor.matmul(p_ws[:], tabw[:, 210:211], tabw[:, 63:126])
            p_ds = psum.tile([1, 63], DT, tag="p_ds")
            nc.tensor.matmul(p_ds[:], tabw[:, 211:212], tabw[:, 126:189])
            p_cs = psum.tile([1, 20], DT, tag="p_cs")
            nc.tensor.matmul(p_cs[:], tabw[:, 212:213], tabw[:, 189:209])

            # scaled copies PSUM -> SBUF (ACT)
            wd_sb = const.tile([1, 146], DT)
            nc.scalar.mul(wd_sb[:, 0:63], p_ws[:], C_CONT)
            nc.scalar.mul(wd_sb[:, 63:126], p_ds[:], C_CONT)
            nc.scalar.mul(wd_sb[:, 126:146], p_cs[:], C_CROSS)
            hs_row = const.tile([1, 31], DT)
            nc.scalar.mul(hs_row[:], p_hs[:], C_CONT)

            # broadcast hs / cross across partitions via ones-matmul on PE
            p_hb = psum.tile([128, 31], DT, tag="p_hb")
            nc.tensor.matmul(p_hb[:], ones[:], hs_row[:])
            p_cb = psum.tile([128, 20], DT, tag="p_cs", name="p_cb")
            nc.tensor.matmul(p_cb[:], ones[:], wd_sb[:, 126:146])

            # Hankel expansion straight from SBUF (no DRAM bounce): one
            # 1-partition overlapping-read SBUF->SBUF DMA per table.
            # H_w[k, a] = ws[16+k+a]*C ; H_d[k, m] = ds[16+k+m]*C
            wa = wd_sb[:]
            h_w = const.tile([16, 16], DT)
            nc.sync.dma_start(h_w[:],
                              bass.AP(wa.tensor, wa.offset + 16,
                                      [[146, 1], [1, 16], [1, 16]]))
            h_d = const.tile([16, 16], DT)
            nc.scalar.dma_start(h_d[:],
                                bass.AP(wa.tensor, wa.offset + 79,
                                        [[146, 1], [1, 16], [1, 16]]))

            hs_sb = const.tile([128, 31], DT)
            nc.scalar.mul(hs_sb[:], p_hb[:], 1.0)
            cs_bc = const.tile([128, 20], DT)
            nc.scalar.mul(cs_bc[:], p_cb[:], 1.0)

            # zero rows 0..19 from the zeroed slot (sync ring, early)
            nc.sync.dma_start(out_d[0:DIMI, :], slots[NSLOT - 1][0:DIMI, :])

            # ---- expand via PE: wexp_b[p, a] = ws[31-i2(p)+a], etc ----
            p_w0 = psum.tile([128, 16], DT, tag="p_hs", name="p_w0")
            nc.tensor.matmul(p_w0[:], a_w0[:], h_w[:])
            p_w1 = psum.tile([128, 16], DT, tag="p_ws", name="p_w1")
            nc.tensor.matmul(p_w1[:], a_w1[:], h_w[:])
            p_d = psum.tile([128, 16], DT, tag="p_ds", name="p_d")
            nc.tensor.matmul(p_d[:], a_d[:], h_d[:])
            wexp0 = const.tile([128, 16], DT)
            nc.scalar.mul(wexp0[:], p_w0[:], 1.0)
            wexp1 = const.tile([128, 16], DT)
            nc.scalar.mul(wexp1[:], p_w1[:], 1.0)
            dexp = const.tile([128, 16], DT)
            nc.scalar.mul(dexp[:], p_d[:], 1.0)

            # cross columns into slots 0..4 (slot 5 after its zeros store)
            for s in range(NSLOT - 1):
                nc.vector.tensor_copy(slots[s][:, 0:DIMI], cs_bc[:])

            # ---- cb halves: cb_b[p, 16a+m] = wexp_b[p, a] + dexp[p, m] ----
            da = dexp[:]
            d_rep = bass.AP(da.tensor, da.offset, [[16, 128], [0, 16], [1, 16]])
            cbs = []
            for b, (wt, eng) in enumerate([(wexp0, nc.vector), (wexp1, nc.gpsimd)]):
                cb = const.tile([128, 256], DT, tag=f"cb{b}", name=f"cb{b}")
                wa = wt[:]
                w_exp = bass.AP(wa.tensor, wa.offset, [[16, 128], [1, 16], [0, 16]])
                eng.tensor_add(cb[:], w_exp, d_rep)
                cbs.append(cb)

            # ---- per-stripe content: ONE DVE broadcast-add into the slot:
            # slot[p, 20 + 256*j1 + m] = cb_b[p, m] + hs_sb[p, 15 - i1 + j1]
            # Measured 4.42 us per add — below the 4.96 us store drain
            # cadence, so the DVE keeps the ring sated. Nothing else runs
            # on vector/pool concurrently (a concurrent big pool tensor_add
            # thrashes DVE: 4.42 -> 11.3 us measured).
            ha = hs_sb[:]

            def stripe_add(k, j0=0, nj=16):
                # blocks j1 = j0 .. j0+nj-1 of stripe k
                i1, b = k >> 1, k & 1
                ca = cbs[b][:]
                cb_rep = bass.AP(ca.tensor, ca.offset, [[256, 128], [0, nj], [1, 256]])
                hs_rep = bass.AP(ha.tensor, ha.offset + 15 - i1 + j0,
                                 [[31, 128], [1, nj], [0, 256]])
                c0 = DIMI + 256 * j0
                nc.vector.tensor_add(slots[k % NSLOT][:, c0:c0 + 256 * nj],
                                     cb_rep, hs_rep)

            def stripe_store(k, c0=0, ncols=DIMQ):
                r0 = DIMI + 128 * k
                st = slots[k % NSLOT][:, c0:c0 + ncols]
                dst = bass.AP(out_d, r0 * DIMQ + c0, [[DIMQ, 128], [1, ncols]])
                nc.sync.dma_start(dst, st)

            # add_k must precede store_k (and follow store_{k-NSLOT}) in
            # PROGRAM order — Tile's dependency tracking is program-order
            # based. The first two stripes are split in half so the ring
            # starts draining ~2 us earlier (it is idle at that point).
            for k in range(2):
                stripe_add(k, 0, 8)
                stripe_store(k, 0, DIMI + 2048)
                stripe_add(k, 8, 8)
                stripe_store(k, DIMI + 2048, 2048)
            for k in range(2, 32):
                if k == 5:
                    # slot5 cross columns (zeros store has completed by now)
                    nc.vector.tensor_copy(slots[NSLOT - 1][:, 0:DIMI], cs_bc[:])
                stripe_add(k)
                stripe_store(k)
    return _split_multi_waits(nc) if split else nc


def kernel(enc_cross, enc_h, enc_w, enc_d, w_cross, w_h, w_w, w_d,
           dim_q=4116, dim_k=4116, dim_i=20, dim_h=16, dim_w=16, dim_d=16,
           **_ignored):
    import os
    from concourse.bass_utils import run_bass_kernel_spmd
    global LAST_RESULTS

    enc_cross = np.asarray(enc_cross, dtype=np.float32)
    enc_h = np.asarray(enc_h, dtype=np.float32)
    enc_w = np.asarray(enc_w, dtype=np.float32)
    enc_d = np.asarray(enc_d, dtype=np.float32)
    w_cross = np.asarray(w_cross, dtype=np.float32)
    w_h = np.asarray(w_h, dtype=np.float32)
    w_w = np.asarray(w_w, dtype=np.float32)
    w_d = np.asarray(w_d, dtype=np.float32)

    cross2d = np.ascontiguousarray(enc_cross.reshape(DIMI, 64))
    tab = np.concatenate([enc_h.T, enc_w.T, enc_d.T, cross2d.T], axis=1)

    if "nc" not in _CACHE:
        _CACHE["nc"] = _build_nc()
    nc = _CACHE["nc"]

    core_ids = list(range(8))
    in_maps = []
    for h in core_ids:
        wp = np.stack([w_h[h], w_w[h], w_d[h], w_cross[h]], axis=1)
        tabw = np.ascontiguousarray(np.concatenate([tab, wp], axis=1))
        in_maps.append({"tabw": tabw})

    trace = bool(int(os.environ.get("KERNEL_TRACE", "0")))
    LAST_RESULTS = run_bass_kernel_spmd(nc, in_maps, core_ids, trace=trace)
    out = np.stack([LAST_RESULTS.results[h]["out"] for h in core_ids], axis=0)
    return out[None]
